# revision 50
# baseline (speedup 1.0000x reference)
"""Causal single-head attention (S=4096, dmodel=1024, dk=128) on 8 TRN2 cores.

Two-launch design: sharded K/V/Q projections, then attention.

Launch 1 (per core c): K^T cols [512c,512c+512), V rows same slice, Q^T for
rows c::8 — each core does 1/8 of the projection FLOPs.  Host gathers the
bf16 K^T/V tiles (1 MB each) and feeds them to launch 2, which runs the
same interleaved-causal attention as the single-launch kernel.
"""

import math

import numpy as np
import ml_dtypes

S = 4096
D = 1024
DK = 128
NCORES = 8
SL = S // NCORES
P = 128
NKC = S // P
DCH = D // P
NCG = S // 512

BF16 = ml_dtypes.bfloat16

_CACHE = {}


def _build_proj():
    import concourse.mybir as mybir
    from concourse import bacc
    from contextlib import ExitStack
    from concourse.tile import TileContext

    f32 = mybir.dt.float32
    bf16 = mybir.dt.bfloat16

    CW = 2 * SL + 3 * DK          # packed row: xs | xq | wk | wv | wq
    nc = bacc.Bacc(None, name="proj")
    allin = nc.dram_tensor("allin", [D, CW], bf16, kind="ExternalInput")
    bias = nc.dram_tensor("bias", [P, DK + 2], f32, kind="ExternalInput")
    kts = nc.dram_tensor("kts", [P, SL], bf16, kind="ExternalOutput")
    vss = nc.dram_tensor("vss", [P, SL], bf16, kind="ExternalOutput")
    qts = nc.dram_tensor("qts", [P, SL], bf16, kind="ExternalOutput")

    with TileContext(nc) as tc, ExitStack() as ctx:
        pool = ctx.enter_context(tc.tile_pool(name="pool", bufs=1))
        psum = ctx.enter_context(tc.tile_pool(name="psum", bufs=3, space="PSUM"))

        ain = pool.tile([P, DCH, CW], bf16)
        ain_r = allin[:, :].rearrange("(c p) w -> p c w", p=P)
        for d in range(DCH):
            nc.sync.dma_start(ain[:, d], ain_r[:, d])
        b_sb = pool.tile([P, DK + 2], f32)
        nc.sync.dma_start(b_sb, bias[:, :])
        xs_sb = ain[:, :, 0:SL]
        xq_sb = ain[:, :, SL:2 * SL]
        wk_sb = ain[:, :, 2 * SL:2 * SL + DK]
        wv_sb = ain[:, :, 2 * SL + DK:2 * SL + 2 * DK]
        wq_sb = ain[:, :, 2 * SL + 2 * DK:2 * SL + 3 * DK]
        bvb_sb = b_sb[:, 0:DK]
        bcol_sb = b_sb[:, DK:DK + 2]

        k_sb = pool.tile([P, SL], bf16)
        v_sb = pool.tile([P, SL], bf16)
        q_sb = pool.tile([P, SL], bf16)
        ones_w = pool.tile([P, 1], bf16)
        nc.vector.memset(ones_w, 1.0)

        # warm-up matmuls spanning the initial input-DMA wait
        psW = psum.tile([P, SL], f32, tag="ps")
        for _ in range(24):
            nc.tensor.matmul(psW[0:1, 0:1], lhsT=ones_w, rhs=ones_w[:, 0:1],
                             start=True, stop=True, skip_group_check=True)

        psK = psum.tile([P, SL], f32, tag="ps")
        for d in range(DCH):
            nc.tensor.matmul(psK, lhsT=wk_sb[:, d], rhs=xs_sb[:, d],
                             start=(d == 0), stop=(d == DCH - 1))
        nc.vector.tensor_scalar_add(k_sb, psK, bcol_sb[:, 0:1])
        nc.sync.dma_start(kts[:, :], k_sb)

        psV = psum.tile([P, SL], f32, tag="ps")
        for t in range(4):
            sl = slice(128 * t, 128 * (t + 1))
            for d in range(DCH):
                nc.tensor.matmul(psV[:, sl], lhsT=xs_sb[:, d, sl], rhs=wv_sb[:, d],
                                 start=(d == 0), stop=(d == DCH - 1))
        nc.vector.tensor_tensor(
            v_sb.rearrange("p (t v) -> p t v", t=4),
            psV.rearrange("p (t v) -> p t v", t=4),
            bvb_sb[:, None, :].to_broadcast((P, 4, DK)),
            mybir.AluOpType.add)
        nc.sync.dma_start(vss[:, :], v_sb)

        psQ = psum.tile([P, SL], f32, tag="ps")
        for d in range(DCH):
            nc.tensor.matmul(psQ, lhsT=wq_sb[:, d], rhs=xq_sb[:, d],
                             start=(d == 0), stop=(d == DCH - 1))
        nc.vector.tensor_scalar_add(q_sb, psQ, bcol_sb[:, 1:2])
        nc.sync.dma_start(qts[:, :], q_sb)

    nc.finalize()
    return nc


def _build_attn():
    import concourse.mybir as mybir
    from concourse import bacc
    from contextlib import ExitStack
    from concourse.tile import TileContext

    f32 = mybir.dt.float32
    bf16 = mybir.dt.bfloat16
    EXP = mybir.ActivationFunctionType.Exp

    nc = bacc.Bacc(None, name="attn")
    qkf = nc.dram_tensor("qkf", [P, SL + S], bf16, kind="ExternalInput")
    vsf = nc.dram_tensor("vsf", [P, S], bf16, kind="ExternalInput")
    mask = nc.dram_tensor("mask", [P, 8 * P], bf16, kind="ExternalInput")
    outT = nc.dram_tensor("outT", [DK, SL], f32, kind="ExternalOutput")
    den = nc.dram_tensor("den", [1, SL], f32, kind="ExternalOutput")

    with TileContext(nc) as tc, ExitStack() as ctx:
        pool = ctx.enter_context(tc.tile_pool(name="pool", bufs=1))
        epool = ctx.enter_context(tc.tile_pool(name="epool", bufs=8))
        pscore = ctx.enter_context(tc.tile_pool(name="pscore", bufs=3, space="PSUM"))
        pacc = ctx.enter_context(tc.tile_pool(name="pacc", bufs=1, space="PSUM"))

        qk_sb = pool.tile([P, SL + S], bf16)
        qT = qk_sb[:, 0:SL]
        kT = qk_sb[:, SL:SL + S]
        vS = pool.tile([P, S], bf16)
        # first load = exactly the first-compute working set (qT + k chunks 0-1),
        # then the mask (needed by block 0's DVE multiply), then k pieces one
        # step ahead of their v counterparts (k is the consumption laggard)
        qk_cuts = [0, SL + 256, SL + 1024, SL + 2048, SL + 3072, SL + S]
        v_cuts = [0, 1024, 2048, 3072, S]
        nc.sync.dma_start(qk_sb[:, qk_cuts[0]:qk_cuts[1]],
                          qkf[:, qk_cuts[0]:qk_cuts[1]])
        msk_sb = pool.tile([P, 8 * P], bf16)
        nc.sync.dma_start(msk_sb, mask[:, :])
        for i in range(1, 5):
            nc.sync.dma_start(qk_sb[:, qk_cuts[i]:qk_cuts[i + 1]],
                              qkf[:, qk_cuts[i]:qk_cuts[i + 1]])
            nc.sync.dma_start(vS[:, v_cuts[i - 1]:v_cuts[i]],
                              vsf[:, v_cuts[i - 1]:v_cuts[i]])
        ones_col = pool.tile([P, 1], bf16)
        nc.vector.memset(ones_col, 1.0)

        psAV = pacc.tile([DK, SL], f32, tag="av")
        psSum = pacc.tile([1, SL], f32, tag="sum")

        # data-independent warm-up matmuls: keep PE busy (and HAM warm)
        # through the initial K/Q DMA wait; first real SUM matmul clears
        # the bank with start=True, so the scratch writes are harmless.
        for _ in range(64):
            nc.tensor.matmul(psSum[0:1, 0:1], lhsT=ones_col, rhs=ones_col[:, 0:1],
                             start=True, stop=True, skip_group_check=True)

        def scores_block(n):
            """Scores + exp + mask for key chunks 2n, 2n+1 (share g0/NW)."""
            g0 = (2 * n) // 8
            qlo = P * g0
            NW = SL - qlo
            w0 = (2 * n) % 8
            psS4 = pscore.tile([P, 2, 512], f32, tag="ps")
            for t in range(2):
                m = 2 * n + t
                nc.tensor.matmul(psS4[:, t, :NW], lhsT=kT[:, P * m:P * (m + 1)],
                                 rhs=qT[:, qlo:], start=True, stop=True,
                                 skip_group_check=True)
            e4 = epool.tile([P, 2, 512], bf16, tag="e")
            nc.scalar.activation(e4[:, :, :NW], psS4[:, :, :NW], EXP)
            nc.vector.tensor_tensor(
                e4[:, :, :P], e4[:, :, :P],
                msk_sb[:, P * w0:P * (w0 + 2)].rearrange("p (t v) -> p t v", t=2),
                mybir.AluOpType.mult)
            return [(2 * n + t, qlo, NW, e4[:, t]) for t in range(2)]

        def av_block(e_tiles):
            for (m, qlo, NW, e_t) in e_tiles:
                nc.tensor.matmul(psAV[:, qlo:], lhsT=vS[:, P * m:P * (m + 1)],
                                 rhs=e_t[:, :NW], start=(m == 0), stop=(m == NKC - 1),
                                 skip_group_check=True)
                nc.tensor.matmul(psSum[:, qlo:], lhsT=ones_col, rhs=e_t[:, :NW],
                                 start=(m == 0), stop=(m == NKC - 1),
                                 skip_group_check=True)

        pend = [scores_block(0), scores_block(1), scores_block(2)]
        for n in range(3, NKC // 2):
            nxt = scores_block(n)
            av_block(pend.pop(0))
            pend.append(nxt)
        for e in pend:
            av_block(e)

        ctx_sb = pool.tile([DK, SL], f32)
        nc.vector.tensor_copy(ctx_sb, psAV)
        den_sb = pool.tile([1, SL], f32)
        nc.vector.tensor_copy(den_sb, psSum)
        nc.sync.dma_start(outT[:, :], ctx_sb)
        nc.sync.dma_start(den[:, :], den_sb)

    nc.finalize()
    return nc


def get_ncs():
    if "nc1" not in _CACHE:
        _CACHE["nc1"] = _build_proj()
        _CACHE["nc2"] = _build_attn()
    return _CACHE["nc1"], _CACHE["nc2"]


def make_in_maps1(inputs):
    X = np.asarray(inputs["X"], np.float32)
    Wq = np.asarray(inputs["Wq"], np.float32)
    bq = np.asarray(inputs["bq"], np.float32)
    Wk = np.asarray(inputs["Wk"], np.float32)
    bk = np.asarray(inputs["bk"], np.float32)
    Wv = np.asarray(inputs["Wv"], np.float32)
    bv = np.asarray(inputs["bv"], np.float32)
    scale = 1.0 / math.sqrt(DK)
    WkT16 = np.ascontiguousarray(Wk.T).astype(BF16)
    WvT16 = np.ascontiguousarray(Wv.T).astype(BF16)
    WqT16 = np.ascontiguousarray((Wq * scale).T).astype(BF16)
    bcolf = np.stack([bk, bq * scale], axis=1).astype(np.float32)
    bvb = np.broadcast_to(bv[None, :], (P, DK)).astype(np.float32)
    XT16 = X.T.astype(BF16)
    bias = np.concatenate([bvb, bcolf], axis=1)          # [P, DK+2] f32
    maps = []
    for c in range(NCORES):
        xst = XT16[:, 512 * c:512 * (c + 1)]
        xqt = X[c::NCORES].T.astype(BF16)
        allin = np.ascontiguousarray(
            np.concatenate([xst, xqt, WkT16, WvT16, WqT16], axis=1))
        maps.append({"allin": allin, "bias": bias})
    return maps


def make_in_maps2(res1):
    ktf = np.concatenate([np.asarray(r["kts"]) for r in res1], axis=1)
    vsf = np.concatenate([np.asarray(r["vss"]) for r in res1], axis=1)
    kr = np.arange(P)[:, None]
    p = np.arange(P)[None, :]
    maps = []
    for c in range(NCORES):
        cols = [((8 * p + c - 128 * w - kr) >= 0) for w in range(8)]
        mask_c = np.concatenate(cols, axis=1).astype(BF16)
        qkf = np.ascontiguousarray(
            np.concatenate([np.asarray(res1[c]["qts"]), ktf], axis=1))
        maps.append({"qkf": qkf, "vsf": vsf, "mask": mask_c})
    return maps


LAST_RESULTS = None


def kernel(**inputs) -> np.ndarray:
    global LAST_RESULTS
    from concourse.bass_utils import run_bass_kernel_spmd

    nc1, nc2 = get_ncs()
    res1 = run_bass_kernel_spmd(nc1, make_in_maps1(inputs),
                                core_ids=list(range(NCORES)))
    res2 = run_bass_kernel_spmd(nc2, make_in_maps2(res1.results),
                                core_ids=list(range(NCORES)))
    LAST_RESULTS = (res1, res2)
    out = np.empty((S, DK), np.float32)
    for c in range(NCORES):
        ctxT = np.asarray(res2.results[c]["outT"], np.float32)
        dnm = np.asarray(res2.results[c]["den"], np.float32)
        out[c::NCORES] = (ctxT / dnm).T
    return out
